# revision 10
# baseline (speedup 1.0000x reference)
"""Multi-head attention (2D-RoPE, masked softmax) on 8 Trainium2 NeuronCores.

Sharding: 4 head-groups (3 heads each) x 2 query-halves (1160 rows each).
Each core computes full attention for its 3 heads over its 1160 query rows
against all 2320 keys, plus its share of the output projection; the host
sums the 8 partial projections and adds the (folded) biases.

Device-side layout notes:
  - Everything PE-facing is float32r (~13-bit mantissa, 1 cyc/row at N>=256).
  - K/Q are produced directly in transposed [head_dim, seq] layout so
    scores come out as scoresT[m, l] with keys on partitions -> softmax
    needs no partition reduction: exp on ScalarE straight out of PSUM
    (0.125 scale folded in; no max-subtraction needed, |s*scale| < ~3),
    denominator via a ones-column appended to V (free PSUM row), and the
    mask folded into V (zeroed rows + mask-valued ones-column).
  - Softmax normalization: Z row -> reciprocal_approx_accurate -> K=1
    outer-product matmul broadcasts 1/Z across partitions -> one DVE
    multiply normalizes ctxT before the output projection.
  - Key order (m) is contraction-internal, so each core's x/K-tables/mask
    are permuted to put its own query rows first; one xT buffer serves
    both the K/V projections (all columns) and the Q projection (first
    1160 columns), keeping the program identical across cores (SPMD).
  - V-bias and output bias never touch the device:
    out = softmax(..) @ (Vx + bv) @ Wp.T + bp = dev_out + (Wp @ bv + bp).
"""
import sys
if '/opt/trn_rl_repo' not in sys.path:
    sys.path.insert(0, '/opt/trn_rl_repo')
import numpy as np

SEQ, E, NH, D = 2320, 768, 12, 64
GRID, TASK = 48, 16
SQ = SEQ // 2           # query rows per core
HG = 3                  # heads per core
SCALE = D ** -0.5
EC = 6                  # embed chunks of 128
L_TILES = [(0, 512), (512, 392), (904, 256)]
N_TILES = [(0, 512), (512, 512), (1024, 512), (1536, 512), (2048, 272)]
MC = [(i * 128, min(128, SEQ - i * 128)) for i in range(19)]
PT = [(i * 128, min(128, SQ - i * 128)) for i in range(10)]

_prog = None


def _build(stages=3):
    import concourse.mybir as mybir
    import concourse.tile as tile
    from concourse import bacc

    F32, F32R = mybir.dt.float32, mybir.dt.float32r
    AF = mybir.ActivationFunctionType

    nc = bacc.Bacc('TRN2', target_bir_lowering=False, debug=False, num_devices=8)
    dp = nc.declare_dram_parameter
    xt_d = dp("xt", [E, SEQ], F32R, isOutput=False)
    wq_d = dp("wq", [E, 192], F32R, isOutput=False)
    wk_d = dp("wk", [E, 192], F32R, isOutput=False)
    wv_d = dp("wv", [E, 256], F32R, isOutput=False)
    wp_d = dp("wp", [192, E], F32R, isOutput=False)
    bq_d = dp("bq", [128, 2], F32, isOutput=False)
    bk_d = dp("bk", [128, 2], F32, isOutput=False)
    cq_d = dp("cq", [128, SQ], F32, isOutput=False)
    sq_d = dp("sq", [128, SQ], F32, isOutput=False)
    ck_d = dp("ck", [128, SEQ], F32, isOutput=False)
    sk_d = dp("sk", [128, SEQ], F32, isOutput=False)
    mk_d = dp("mk", [19, 128], F32, isOutput=False)
    on_d = dp("ones64", [1, 64], F32R, isOutput=False)
    out_d = dp("pout", [SQ, E], F32, isOutput=True)

    with tile.TileContext(nc) as tc:
        with (
            tc.tile_pool(name="long", bufs=1) as lp,
            tc.tile_pool(name="zp", bufs=2) as zp,
        ):
            kt01 = lp.tile([128, SEQ], F32R, tag="kt01")
            kt2 = lp.tile([64, SEQ], F32R, tag="kt2")
            qt01 = lp.tile([128, SQ], F32R, tag="qt01")
            qt2 = lp.tile([64, SQ], F32R, tag="qt2")
            v_all = lp.tile([128, 19, HG, 65], F32R, tag="v_all")
            wp_h = [lp.tile([64, E], F32R, tag=f"wp{h}", name=f"wp{h}") for h in range(HG)]
            ctxn_h = [lp.tile([64, SQ], F32R, tag=f"ctxn{h}", name=f"ctxn{h}") for h in range(HG)]
            ones64 = lp.tile([1, 64], F32R, tag="ones64")
            mk_sb = lp.tile([128, 19], F32, tag="mk")

            for h in range(HG):
                nc.sync.dma_start(wp_h[h][:], wp_d[64 * h:64 * (h + 1), :])
            nc.sync.dma_start(ones64[:], on_d[:])
            nc.sync.dma_start(mk_sb[:], mk_d[:].rearrange("t p -> p t"))

            with tc.tile_pool(name="p12", bufs=1) as p12, \
                 tc.tile_pool(name="rp", bufs=2) as rp, \
                 tc.tile_pool(name="pk", bufs=2, space="PSUM") as pkp, \
                 tc.tile_pool(name="pv", bufs=2, space="PSUM") as pvp:
                xt = []
                for c in range(EC):
                    t = p12.tile([128, SEQ], F32R, tag=f"xt{c}", name=f"xt{c}")
                    for (off, n) in N_TILES:
                        nc.sync.dma_start(t[:, off:off + n],
                                          xt_d[c * 128:(c + 1) * 128, off:off + n])
                    xt.append(t)
                wq_sb = p12.tile([128, EC, 192], F32R, tag="wq")
                nc.sync.dma_start(wq_sb[:], wq_d[:].rearrange("(c p) n -> p c n", p=128))
                wk_sb = p12.tile([128, EC, 192], F32R, tag="wk")
                nc.sync.dma_start(wk_sb[:], wk_d[:].rearrange("(c p) n -> p c n", p=128))
                wv_sb = p12.tile([128, EC, 256], F32R, tag="wv")
                nc.sync.dma_start(wv_sb[:], wv_d[:].rearrange("(c p) n -> p c n", p=128))
                bq_sb = p12.tile([128, 2], F32, tag="bq")
                nc.sync.dma_start(bq_sb[:], bq_d[:])
                bk_sb = p12.tile([128, 2], F32, tag="bk")
                nc.sync.dma_start(bk_sb[:], bk_d[:])
                cq_sb = p12.tile([128, SQ], F32, tag="cq")
                nc.sync.dma_start(cq_sb[:], cq_d[:])
                sq_sb = p12.tile([128, SQ], F32, tag="sq")
                nc.sync.dma_start(sq_sb[:], sq_d[:])
                ck_sb = p12.tile([128, SEQ], F32, tag="ck")
                nc.sync.dma_start(ck_sb[:], ck_d[:])
                sk_sb = p12.tile([128, SEQ], F32, tag="sk")
                nc.sync.dma_start(sk_sb[:], sk_d[:])

                def qk_proj(w_sb, b_sb, cos_sb, sin_sb, tiles, out01, out2):
                    # pair (h0,h1) stacked on 128 partitions, then h2 alone.
                    # fp32r matmuls cannot col-tile (dst base must be 0), so
                    # each head projects into its own base-0 psum tile and the
                    # bias-copy stacks them into raw.
                    for grp in (0, 1):
                        P = 128 if grp == 0 else 64
                        heads = (0, 1) if grp == 0 else (2,)
                        bcol = 0 if grp == 0 else 1
                        for (off, n) in tiles:
                            raw = rp.tile([128, 512], F32, tag="raw")
                            for j, h in enumerate(heads):
                                ps = pkp.tile([64, 512], F32, tag=f"pk{j}")
                                for c in range(EC):
                                    nc.tensor.matmul(
                                        ps[0:64, 0:n],
                                        w_sb[:, c, h * 64:(h + 1) * 64],
                                        xt[c][:, off:off + n],
                                        start=(c == 0), stop=(c == EC - 1),
                                    )
                                ob = j * 64
                                brow = ob if grp == 0 else 0
                                nc.vector.tensor_scalar_add(
                                    raw[ob:ob + 64, 0:n], ps[0:64, 0:n],
                                    b_sb[brow:brow + 64, bcol:bcol + 1])
                            t1 = rp.tile([128, 512], F32, tag="rt1")
                            t2 = rp.tile([128, 512], F32, tag="rt2")
                            nc.vector.tensor_mul(
                                t1[0:P, 0:n], raw[0:P, 0:n], cos_sb[0:P, off:off + n])
                            for b in range(P // 32):
                                src = b * 32 + (32 if b % 2 == 0 else -32)
                                nc.vector.tensor_mul(
                                    t2[b * 32:(b + 1) * 32, 0:n],
                                    raw[src:src + 32, 0:n],
                                    sin_sb[src:src + 32, off:off + n])
                            outt = out01 if grp == 0 else out2
                            nc.vector.tensor_add(
                                outt[0:P, off:off + n], t1[0:P, 0:n], t2[0:P, 0:n])

                qk_proj(wk_sb, bk_sb, ck_sb, sk_sb, N_TILES, kt01, kt2)
                qk_proj(wq_sb, bq_sb, cq_sb, sq_sb, L_TILES, qt01, qt2)

                # V projection (padded to N=256 for fp32r full rate),
                # mask folded in: value rows *= mask, ones-col = mask
                for i, (off, m) in enumerate(MC):
                    pv = pvp.tile([128, 256], F32, tag="pv")
                    for c in range(EC):
                        nc.tensor.matmul(
                            pv[0:m, :], xt[c][:, off:off + m], wv_sb[:, c, :],
                            start=(c == 0), stop=(c == EC - 1))
                    nc.vector.tensor_mul(
                        v_all[0:m, i, :, 0:64],
                        pv[0:m, 0:192].rearrange("p (h d) -> p h d", h=HG),
                        mk_sb[0:m, i:i + 1].to_broadcast([m, HG, 64]))
                    nc.vector.tensor_copy(
                        v_all[0:m, i, :, 64:65],
                        mk_sb[0:m, i:i + 1].to_broadcast([m, HG, 1]))

            if stages < 2:
                nc.sync.dma_start(out_d[0:128, :],
                                  kt01[0:128, 0:768].bitcast(F32))
            # ---- attention phase ----
            with tc.tile_pool(name="p3", bufs=1) as p3, \
                 tc.tile_pool(name="ep", bufs=4) as ep, \
                 tc.tile_pool(name="op", bufs=2) as op, \
                 tc.tile_pool(name="rzp", bufs=2) as rzp, \
                 tc.tile_pool(name="ps3", bufs=4, space="PSUM") as ps3, \
                 tc.tile_pool(name="pc3", bufs=2, space="PSUM") as pc3, \
                 tc.tile_pool(name="pz3", bufs=1, space="PSUM") as pz3, \
                 tc.tile_pool(name="pp3", bufs=1, space="PSUM") as pp3:
                for h in range(HG if stages >= 2 else 0):
                    if h < 2:
                        ktap, qtap, rb = kt01, qt01, (h % 2) * 64
                    else:
                        ktap, qtap, rb = kt2, qt2, 0
                    ctxap = ctxn_h[h]
                    for (loff, ln) in L_TILES:
                        pctx = pc3.tile([65, 512], F32, tag="pctx")
                        # software-pipelined (skew 2): the ctx matmul for
                        # chunk i issues after scores for i+2, so the exp on
                        # ScalarE overlaps PE work instead of stalling it.
                        SKEW = 2
                        exs = {}

                        def scores_exp(i):
                            moff, m = MC[i]
                            ps = ps3.tile([128, 512], F32, tag="ps", name="ps")
                            nc.tensor.matmul(
                                ps[0:m, 0:ln],
                                ktap[rb:rb + 64, moff:moff + m],
                                qtap[rb:rb + 64, loff:loff + ln],
                                start=True, stop=True)
                            ex = ep.tile([128, 512], F32R, tag="ex", name="ex")
                            nc.scalar.activation(
                                ex[0:m, 0:ln], ps[0:m, 0:ln], AF.Exp,
                                bias=0.0, scale=SCALE)
                            exs[i] = ex

                        def ctx_mm(i):
                            moff, m = MC[i]
                            nc.tensor.matmul(
                                pctx[:, 0:ln], v_all[0:m, i, h, :],
                                exs.pop(i)[0:m, 0:ln],
                                start=(i == 0), stop=(i == len(MC) - 1))

                        for i in range(len(MC) + SKEW):
                            if i < len(MC):
                                scores_exp(i)
                            if i >= SKEW:
                                ctx_mm(i - SKEW)
                        # softmax denominator -> broadcast -> normalize
                        zrow = zp.tile([1, 512], F32, tag="zrow")
                        nc.scalar.copy(zrow[0:1, 0:ln], pctx[64:65, 0:ln])
                        zscr = zp.tile([1, 512], F32, tag="zscr")
                        rzf = zp.tile([1, 512], F32, tag="rzf")
                        nc.vector.reciprocal_approx_accurate(
                            rzf[0:1, 0:ln], zrow[0:1, 0:ln], zscr[0:1, 0:ln])
                        rzr = zp.tile([1, 512], F32R, tag="rzr")
                        nc.vector.tensor_copy(rzr[0:1, 0:ln], rzf[0:1, 0:ln])
                        przb = pz3.tile([64, 512], F32, tag="przb")
                        nc.tensor.matmul(
                            przb[:, 0:ln], ones64[:], rzr[0:1, 0:ln],
                            start=True, stop=True)
                        rzb = rzp.tile([64, 512], F32, tag="rzb")
                        nc.scalar.copy(rzb[:, 0:ln], przb[:, 0:ln])
                        nc.vector.tensor_mul(
                            ctxap[0:64, loff:loff + ln],
                            pctx[0:64, 0:ln], rzb[0:64, 0:ln])

                if stages == 2:
                    nc.sync.dma_start(out_d[0:64, :],
                                      ctxn_h[0][0:64, 0:768].bitcast(F32))
                # output projection: 3 head-chunks accumulate per l-slice
                for (toff, tm) in (PT if stages >= 3 else []):
                    outsb = op.tile([128, E], F32, tag="outsb")
                    for half in range(2):
                        hs = half * 384
                        pp = pp3.tile([128, 384], F32, tag="pp")
                        for hh in range(HG):
                            nc.tensor.matmul(
                                pp[0:tm, :], ctxn_h[hh][0:64, toff:toff + tm],
                                wp_h[hh][:, hs:hs + 384],
                                start=(hh == 0), stop=(hh == HG - 1))
                        nc.vector.tensor_copy(outsb[0:tm, hs:hs + 384], pp[0:tm, :])
                    nc.sync.dma_start(out_d[toff:toff + tm, :], outsb[0:tm, :])
    nc.finalize()
    return nc


def _rope_tables():
    dim = D // 2
    freqs = 1.0 / 10000 ** (np.arange(0, dim, 2, dtype=np.float64) / dim)
    t = np.arange(GRID, dtype=np.float64)
    f = np.repeat(np.outer(t, freqs), 2, axis=-1)                  # [48, 32]
    fr = np.broadcast_to(f[:, None, :], (GRID, GRID, dim))
    fc = np.broadcast_to(f[None, :, :], (GRID, GRID, dim))
    full = np.concatenate([fr, fc], axis=-1).reshape(GRID * GRID, D)
    cos = np.ones((SEQ, D), np.float64)
    sin = np.zeros((SEQ, D), np.float64)
    cos[TASK:] = np.cos(full)
    sin[TASK:] = np.sin(full)
    return cos.astype(np.float32), sin.astype(np.float32)


def _signed_stack(tT):
    # [64, S] -> [128, S]: signed sine table stored at the ROTATED (source)
    # rows, so the rope half-multiplies read both operands at equal partition
    # bases: sinB[32:64] = -sin[0:32], sinB[0:32] = +sin[32:64], stacked x2.
    s = np.vstack([tT[32:64], -tT[0:32]])
    return np.ascontiguousarray(np.vstack([s, s]))


def _core_inputs(x, mask, Wqkv, Wproj, bqkv, cos, sin, g, s):
    xT = x.T  # [768, 2320]
    q0 = SQ * s
    if s == 0:
        perm = None
        xt = np.ascontiguousarray(xT)
    else:
        perm = np.concatenate([np.arange(SQ, SEQ), np.arange(0, SQ)])
        xt = np.ascontiguousarray(np.concatenate([xT[:, SQ:], xT[:, :SQ]], axis=1))
    r0 = 192 * g
    wq = np.ascontiguousarray(Wqkv[r0:r0 + 192, :].T)
    wk = np.ascontiguousarray(Wqkv[768 + r0:768 + r0 + 192, :].T)
    wv = np.zeros((E, 256), np.float32)
    wv[:, 0:192] = Wqkv[1536 + r0:1536 + r0 + 192, :].T
    wp = np.ascontiguousarray(Wproj[:, r0:r0 + 192].T)
    bq = np.zeros((128, 2), np.float32)
    bq[:, 0] = bqkv[r0:r0 + 128]
    bq[0:64, 1] = bqkv[r0 + 128:r0 + 192]
    bk = np.zeros((128, 2), np.float32)
    bk[:, 0] = bqkv[768 + r0:768 + r0 + 128]
    bk[0:64, 1] = bqkv[768 + r0 + 128:768 + r0 + 192]
    cosT, sinT = cos.T, sin.T  # [64, S]
    cq = np.ascontiguousarray(np.vstack([cosT, cosT])[:, q0:q0 + SQ])
    sq = np.ascontiguousarray(_signed_stack(sinT)[:, q0:q0 + SQ])
    ckf = np.vstack([cosT, cosT])
    skf = _signed_stack(sinT)
    if perm is not None:
        ckf = ckf[:, perm]
        skf = skf[:, perm]
    mk = mask.astype(np.float32)
    if perm is not None:
        mk = mk[perm]
    mk = np.concatenate([mk, np.zeros(19 * 128 - SEQ, np.float32)]).reshape(19, 128)
    return {
        "xt": xt, "wq": wq, "wk": wk, "wv": wv, "wp": wp,
        "bq": bq, "bk": bk,
        "cq": cq, "sq": sq,
        "ck": np.ascontiguousarray(ckf), "sk": np.ascontiguousarray(skf),
        "mk": np.ascontiguousarray(mk),
        "ones64": np.ones((1, 64), np.float32),
    }


def _run(x, mask, Wqkv, bqkv, Wproj, bproj, trace=False):
    global _prog
    from concourse.bass_utils import run_bass_kernel_spmd
    if _prog is None:
        _prog = _build()
    x = np.asarray(x, np.float32)
    mask = np.asarray(mask)
    Wqkv = np.asarray(Wqkv, np.float32)
    bqkv = np.asarray(bqkv, np.float32)
    Wproj = np.asarray(Wproj, np.float32)
    bproj = np.asarray(bproj, np.float32)
    cos, sin = _rope_tables()
    in_maps = [
        _core_inputs(x, mask, Wqkv, Wproj, bqkv, cos, sin, core // 2, core % 2)
        for core in range(8)
    ]
    res = run_bass_kernel_spmd(_prog, in_maps, list(range(8)), trace=trace)
    acc = np.zeros((SEQ, E), np.float64)
    for core in range(8):
        s = core % 2
        acc[SQ * s:SQ * (s + 1)] += res.results[core]["pout"].astype(np.float64)
    bias_row = bproj.astype(np.float64) + Wproj.astype(np.float64) @ \
        bqkv[1536:2304].astype(np.float64)
    acc += bias_row
    return acc.astype(np.float32), res


def kernel(x, mask, Wqkv, bqkv, Wproj, bproj):
    out, _ = _run(x, mask, Wqkv, bqkv, Wproj, bproj, trace=False)
    return out


# revision 12
# speedup vs baseline: 1.2921x; 1.2921x over previous
"""Multi-head attention (2D-RoPE, masked softmax) on 8 Trainium2 NeuronCores.

Sharding: 4 head-groups (3 heads each) x 2 query-halves (1160 rows each).
Each core computes full attention for its 3 heads over its 1160 query rows
against all 2320 keys, plus its share of the output projection; the host
sums the 8 partial projections and adds the (folded) biases.

Device-side layout notes:
  - Everything PE-facing is float32r (~13-bit mantissa, 1 cyc/row at N>=256).
  - K/Q are produced directly in transposed [head_dim, seq] layout so
    scores come out as scoresT[m, l] with keys on partitions -> softmax
    needs no partition reduction: exp on ScalarE straight out of PSUM
    (0.125 scale folded in; no max-subtraction needed, |s*scale| < ~3),
    denominator via a ones-column appended to V (free PSUM row), and the
    mask folded into V (zeroed rows + mask-valued ones-column).
  - Softmax normalization: Z row -> reciprocal_approx_accurate -> K=1
    outer-product matmul broadcasts 1/Z across partitions -> one DVE
    multiply normalizes ctxT before the output projection.
  - Key order (m) is contraction-internal, so each core's x/K-tables/mask
    are permuted to put its own query rows first; one xT buffer serves
    both the K/V projections (all columns) and the Q projection (first
    1160 columns), keeping the program identical across cores (SPMD).
  - V-bias and output bias never touch the device:
    out = softmax(..) @ (Vx + bv) @ Wp.T + bp = dev_out + (Wp @ bv + bp).
"""
import sys
if '/opt/trn_rl_repo' not in sys.path:
    sys.path.insert(0, '/opt/trn_rl_repo')
import numpy as np

SEQ, E, NH, D = 2320, 768, 12, 64
GRID, TASK = 48, 16
SQ = SEQ // 2           # query rows per core
HG = 3                  # heads per core
SCALE = D ** -0.5
EC = 6                  # embed chunks of 128
L_TILES = [(0, 512), (512, 392), (904, 256)]
N_TILES = [(0, 512), (512, 512), (1024, 512), (1536, 512), (2048, 272)]
MC = [(i * 128, min(128, SEQ - i * 128)) for i in range(19)]
PT = [(i * 128, min(128, SQ - i * 128)) for i in range(10)]

_prog = None


def _build(stages=3):
    import concourse.mybir as mybir
    import concourse.tile as tile
    from concourse import bacc

    F32, F32R = mybir.dt.float32, mybir.dt.float32r
    AF = mybir.ActivationFunctionType

    nc = bacc.Bacc('TRN2', target_bir_lowering=False, debug=False, num_devices=8)
    dp = nc.declare_dram_parameter
    xt_d = dp("xt", [E, SEQ], F32R, isOutput=False)
    wq_d = dp("wq", [E, 192], F32R, isOutput=False)
    wk_d = dp("wk", [E, 192], F32R, isOutput=False)
    wv_d = dp("wv", [E, 256], F32R, isOutput=False)
    wp_d = dp("wp", [192, E], F32R, isOutput=False)
    bq_d = dp("bq", [128, 2], F32, isOutput=False)
    bk_d = dp("bk", [128, 2], F32, isOutput=False)
    cq_d = dp("cq", [128, SQ], F32, isOutput=False)
    sq_d = dp("sq", [128, SQ], F32, isOutput=False)
    ck_d = dp("ck", [128, SEQ], F32, isOutput=False)
    sk_d = dp("sk", [128, SEQ], F32, isOutput=False)
    mk_d = dp("mk", [19, 128], F32, isOutput=False)
    on_d = dp("ones64", [1, 64], F32R, isOutput=False)
    out_d = dp("pout", [SQ, E], F32, isOutput=True)

    with tile.TileContext(nc) as tc:
        with (
            tc.tile_pool(name="long", bufs=1) as lp,
            tc.tile_pool(name="zp", bufs=2) as zp,
        ):
            kt01 = lp.tile([128, SEQ], F32R, tag="kt01")
            kt2 = lp.tile([64, SEQ], F32R, tag="kt2")
            qt01 = lp.tile([128, SQ], F32R, tag="qt01")
            qt2 = lp.tile([64, SQ], F32R, tag="qt2")
            v_all = lp.tile([128, 19, HG, 65], F32R, tag="v_all")
            wp_h = [lp.tile([64, E], F32R, tag=f"wp{h}", name=f"wp{h}") for h in range(HG)]
            ctxn_h = [lp.tile([64, SQ], F32R, tag=f"ctxn{h}", name=f"ctxn{h}") for h in range(HG)]
            ones64 = lp.tile([1, 64], F32R, tag="ones64")
            mk_sb = lp.tile([128, 19], F32, tag="mk")

            for h in range(HG):
                nc.sync.dma_start(wp_h[h][:], wp_d[64 * h:64 * (h + 1), :])
            nc.sync.dma_start(ones64[:], on_d[:])
            nc.sync.dma_start(mk_sb[:], mk_d[:].rearrange("t p -> p t"))

            with tc.tile_pool(name="p12", bufs=1) as p12, \
                 tc.tile_pool(name="rp", bufs=2) as rp, \
                 tc.tile_pool(name="pk", bufs=2, space="PSUM") as pkp, \
                 tc.tile_pool(name="pv", bufs=2, space="PSUM") as pvp:
                xt = []
                for c in range(EC):
                    t = p12.tile([128, SEQ], F32R, tag=f"xt{c}", name=f"xt{c}")
                    for (off, n) in N_TILES:
                        nc.sync.dma_start(t[:, off:off + n],
                                          xt_d[c * 128:(c + 1) * 128, off:off + n])
                    xt.append(t)
                wq_sb = p12.tile([128, EC, 192], F32R, tag="wq")
                nc.sync.dma_start(wq_sb[:], wq_d[:].rearrange("(c p) n -> p c n", p=128))
                wk_sb = p12.tile([128, EC, 192], F32R, tag="wk")
                nc.sync.dma_start(wk_sb[:], wk_d[:].rearrange("(c p) n -> p c n", p=128))
                wv_sb = p12.tile([128, EC, 256], F32R, tag="wv")
                nc.sync.dma_start(wv_sb[:], wv_d[:].rearrange("(c p) n -> p c n", p=128))
                bq_sb = p12.tile([128, 2], F32, tag="bq")
                nc.sync.dma_start(bq_sb[:], bq_d[:])
                bk_sb = p12.tile([128, 2], F32, tag="bk")
                nc.sync.dma_start(bk_sb[:], bk_d[:])
                cq_sb = p12.tile([128, SQ], F32, tag="cq")
                nc.sync.dma_start(cq_sb[:], cq_d[:])
                sq_sb = p12.tile([128, SQ], F32, tag="sq")
                nc.sync.dma_start(sq_sb[:], sq_d[:])
                ck_sb = p12.tile([128, SEQ], F32, tag="ck")
                nc.sync.dma_start(ck_sb[:], ck_d[:])
                sk_sb = p12.tile([128, SEQ], F32, tag="sk")
                nc.sync.dma_start(sk_sb[:], sk_d[:])

                def qk_proj(w_sb, b_sb, cos_sb, sin_sb, tiles, out01, out2):
                    # pair (h0,h1) stacked on 128 partitions, then h2 alone.
                    # fp32r matmuls cannot col-tile (dst base must be 0), so
                    # each head projects into its own base-0 psum tile and the
                    # bias-copy stacks them into raw.
                    for grp in (0, 1):
                        P = 128 if grp == 0 else 64
                        heads = (0, 1) if grp == 0 else (2,)
                        bcol = 0 if grp == 0 else 1
                        for (off, n) in tiles:
                            raw = rp.tile([128, 512], F32, tag="raw")
                            for j, h in enumerate(heads):
                                ps = pkp.tile([64, 512], F32, tag=f"pk{j}")
                                for c in range(EC):
                                    nc.tensor.matmul(
                                        ps[0:64, 0:n],
                                        w_sb[:, c, h * 64:(h + 1) * 64],
                                        xt[c][:, off:off + n],
                                        start=(c == 0), stop=(c == EC - 1),
                                    )
                                ob = j * 64
                                brow = ob if grp == 0 else 0
                                nc.vector.tensor_scalar_add(
                                    raw[ob:ob + 64, 0:n], ps[0:64, 0:n],
                                    b_sb[brow:brow + 64, bcol:bcol + 1])
                            t1 = rp.tile([128, 512], F32, tag="rt1")
                            t2 = rp.tile([128, 512], F32, tag="rt2")
                            nc.vector.tensor_mul(
                                t1[0:P, 0:n], raw[0:P, 0:n], cos_sb[0:P, off:off + n])
                            for b in range(P // 32):
                                src = b * 32 + (32 if b % 2 == 0 else -32)
                                nc.vector.tensor_mul(
                                    t2[b * 32:(b + 1) * 32, 0:n],
                                    raw[src:src + 32, 0:n],
                                    sin_sb[src:src + 32, off:off + n])
                            outt = out01 if grp == 0 else out2
                            nc.vector.tensor_add(
                                outt[0:P, off:off + n], t1[0:P, 0:n], t2[0:P, 0:n])

                qk_proj(wk_sb, bk_sb, ck_sb, sk_sb, N_TILES, kt01, kt2)
                qk_proj(wq_sb, bq_sb, cq_sb, sq_sb, L_TILES, qt01, qt2)

                # V projection (padded to N=256 for fp32r full rate),
                # mask folded in: value rows *= mask, ones-col = mask
                for i, (off, m) in enumerate(MC):
                    pv = pvp.tile([128, 256], F32, tag="pv")
                    for c in range(EC):
                        nc.tensor.matmul(
                            pv[0:m, :], xt[c][:, off:off + m], wv_sb[:, c, :],
                            start=(c == 0), stop=(c == EC - 1))
                    nc.vector.tensor_mul(
                        v_all[0:m, i, :, 0:64],
                        pv[0:m, 0:192].rearrange("p (h d) -> p h d", h=HG),
                        mk_sb[0:m, i:i + 1].to_broadcast([m, HG, 64]))
                    nc.vector.tensor_copy(
                        v_all[0:m, i, :, 64:65],
                        mk_sb[0:m, i:i + 1].to_broadcast([m, HG, 1]))

            if stages < 2:
                nc.sync.dma_start(out_d[0:128, :],
                                  kt01[0:128, 0:768].bitcast(F32))
            # ---- attention phase ----
            with tc.tile_pool(name="p3", bufs=1) as p3, \
                 tc.tile_pool(name="ep", bufs=2) as ep, \
                 tc.tile_pool(name="op", bufs=2) as op, \
                 tc.tile_pool(name="rzp", bufs=2) as rzp, \
                 tc.tile_pool(name="ps3", bufs=2, space="PSUM") as ps3, \
                 tc.tile_pool(name="pc3", bufs=2, space="PSUM") as pc3, \
                 tc.tile_pool(name="pz3", bufs=1, space="PSUM") as pz3, \
                 tc.tile_pool(name="pp3", bufs=1, space="PSUM") as pp3:
                for h in range(HG if stages >= 2 else 0):
                    if h < 2:
                        ktap, qtap, rb = kt01, qt01, (h % 2) * 64
                    else:
                        ktap, qtap, rb = kt2, qt2, 0
                    ctxap = ctxn_h[h]
                    for (loff, ln) in L_TILES:
                        pctx = pc3.tile([65, 512], F32, tag="pctx")
                        # software-pipelined over PAIRS of m-chunks: one exp
                        # op covers two chunks (striding across two psum
                        # banks), halving ScalarE op count; ctx matmuls for
                        # pair p issue after scores of pair p+1, so exp
                        # overlaps PE work.
                        PAIRS = [(i, i + 1) if i + 1 < len(MC) else (i,)
                                 for i in range(0, len(MC), 2)]
                        exs = {}

                        def scores_exp(p):
                            chunks = PAIRS[p]
                            ps = ps3.tile([128, 1024], F32, tag="ps", name="ps")
                            for j, i in enumerate(chunks):
                                moff, m = MC[i]
                                nc.tensor.matmul(
                                    ps[0:m, j * 512:j * 512 + ln],
                                    ktap[rb:rb + 64, moff:moff + m],
                                    qtap[rb:rb + 64, loff:loff + ln],
                                    start=True, stop=True)
                            ex = ep.tile([128, 2, 512], F32R, tag="ex", name="ex")
                            m0 = MC[chunks[0]][1]
                            if len(chunks) == 2:
                                nc.scalar.activation(
                                    ex[0:m0, :, 0:ln],
                                    ps[0:m0, :].rearrange(
                                        "p (two n) -> p two n", two=2)[:, :, 0:ln],
                                    AF.Exp, bias=0.0, scale=SCALE)
                            else:
                                nc.scalar.activation(
                                    ex[0:m0, 0, 0:ln], ps[0:m0, 0:ln], AF.Exp,
                                    bias=0.0, scale=SCALE)
                            exs[p] = ex

                        def ctx_mm(p):
                            ex = exs.pop(p)
                            for j, i in enumerate(PAIRS[p]):
                                moff, m = MC[i]
                                nc.tensor.matmul(
                                    pctx[:, 0:ln], v_all[0:m, i, h, :],
                                    ex[0:m, j, 0:ln],
                                    start=(i == 0), stop=(i == len(MC) - 1))

                        for p in range(len(PAIRS) + 1):
                            if p < len(PAIRS):
                                scores_exp(p)
                            if p >= 1:
                                ctx_mm(p - 1)
                        # softmax denominator -> broadcast -> normalize
                        zrow = zp.tile([1, 512], F32, tag="zrow")
                        nc.scalar.copy(zrow[0:1, 0:ln], pctx[64:65, 0:ln])
                        zscr = zp.tile([1, 512], F32, tag="zscr")
                        rzf = zp.tile([1, 512], F32, tag="rzf")
                        nc.vector.reciprocal_approx_accurate(
                            rzf[0:1, 0:ln], zrow[0:1, 0:ln], zscr[0:1, 0:ln])
                        rzr = zp.tile([1, 512], F32R, tag="rzr")
                        nc.vector.tensor_copy(rzr[0:1, 0:ln], rzf[0:1, 0:ln])
                        przb = pz3.tile([64, 512], F32, tag="przb")
                        nc.tensor.matmul(
                            przb[:, 0:ln], ones64[:], rzr[0:1, 0:ln],
                            start=True, stop=True)
                        rzb = rzp.tile([64, 512], F32, tag="rzb")
                        nc.scalar.copy(rzb[:, 0:ln], przb[:, 0:ln])
                        nc.vector.tensor_mul(
                            ctxap[0:64, loff:loff + ln],
                            pctx[0:64, 0:ln], rzb[0:64, 0:ln])

                if stages == 2:
                    nc.sync.dma_start(out_d[0:64, :],
                                      ctxn_h[0][0:64, 0:768].bitcast(F32))
                # output projection: 3 head-chunks accumulate per l-slice
                for (toff, tm) in (PT if stages >= 3 else []):
                    outsb = op.tile([128, E], F32, tag="outsb")
                    for half in range(2):
                        hs = half * 384
                        pp = pp3.tile([128, 384], F32, tag="pp")
                        for hh in range(HG):
                            nc.tensor.matmul(
                                pp[0:tm, :], ctxn_h[hh][0:64, toff:toff + tm],
                                wp_h[hh][:, hs:hs + 384],
                                start=(hh == 0), stop=(hh == HG - 1))
                        nc.vector.tensor_copy(outsb[0:tm, hs:hs + 384], pp[0:tm, :])
                    nc.sync.dma_start(out_d[toff:toff + tm, :], outsb[0:tm, :])
    nc.finalize()
    return nc


def _rope_tables():
    dim = D // 2
    freqs = 1.0 / 10000 ** (np.arange(0, dim, 2, dtype=np.float64) / dim)
    t = np.arange(GRID, dtype=np.float64)
    f = np.repeat(np.outer(t, freqs), 2, axis=-1)                  # [48, 32]
    fr = np.broadcast_to(f[:, None, :], (GRID, GRID, dim))
    fc = np.broadcast_to(f[None, :, :], (GRID, GRID, dim))
    full = np.concatenate([fr, fc], axis=-1).reshape(GRID * GRID, D)
    cos = np.ones((SEQ, D), np.float64)
    sin = np.zeros((SEQ, D), np.float64)
    cos[TASK:] = np.cos(full)
    sin[TASK:] = np.sin(full)
    return cos.astype(np.float32), sin.astype(np.float32)


def _signed_stack(tT):
    # [64, S] -> [128, S]: signed sine table stored at the ROTATED (source)
    # rows, so the rope half-multiplies read both operands at equal partition
    # bases: sinB[32:64] = -sin[0:32], sinB[0:32] = +sin[32:64], stacked x2.
    s = np.vstack([tT[32:64], -tT[0:32]])
    return np.ascontiguousarray(np.vstack([s, s]))


def _core_inputs(x, mask, Wqkv, Wproj, bqkv, cos, sin, g, s):
    xT = x.T  # [768, 2320]
    q0 = SQ * s
    if s == 0:
        perm = None
        xt = np.ascontiguousarray(xT)
    else:
        perm = np.concatenate([np.arange(SQ, SEQ), np.arange(0, SQ)])
        xt = np.ascontiguousarray(np.concatenate([xT[:, SQ:], xT[:, :SQ]], axis=1))
    r0 = 192 * g
    wq = np.ascontiguousarray(Wqkv[r0:r0 + 192, :].T)
    wk = np.ascontiguousarray(Wqkv[768 + r0:768 + r0 + 192, :].T)
    wv = np.zeros((E, 256), np.float32)
    wv[:, 0:192] = Wqkv[1536 + r0:1536 + r0 + 192, :].T
    wp = np.ascontiguousarray(Wproj[:, r0:r0 + 192].T)
    bq = np.zeros((128, 2), np.float32)
    bq[:, 0] = bqkv[r0:r0 + 128]
    bq[0:64, 1] = bqkv[r0 + 128:r0 + 192]
    bk = np.zeros((128, 2), np.float32)
    bk[:, 0] = bqkv[768 + r0:768 + r0 + 128]
    bk[0:64, 1] = bqkv[768 + r0 + 128:768 + r0 + 192]
    cosT, sinT = cos.T, sin.T  # [64, S]
    cq = np.ascontiguousarray(np.vstack([cosT, cosT])[:, q0:q0 + SQ])
    sq = np.ascontiguousarray(_signed_stack(sinT)[:, q0:q0 + SQ])
    ckf = np.vstack([cosT, cosT])
    skf = _signed_stack(sinT)
    if perm is not None:
        ckf = ckf[:, perm]
        skf = skf[:, perm]
    mk = mask.astype(np.float32)
    if perm is not None:
        mk = mk[perm]
    mk = np.concatenate([mk, np.zeros(19 * 128 - SEQ, np.float32)]).reshape(19, 128)
    return {
        "xt": xt, "wq": wq, "wk": wk, "wv": wv, "wp": wp,
        "bq": bq, "bk": bk,
        "cq": cq, "sq": sq,
        "ck": np.ascontiguousarray(ckf), "sk": np.ascontiguousarray(skf),
        "mk": np.ascontiguousarray(mk),
        "ones64": np.ones((1, 64), np.float32),
    }


def _run(x, mask, Wqkv, bqkv, Wproj, bproj, trace=False):
    global _prog
    from concourse.bass_utils import run_bass_kernel_spmd
    if _prog is None:
        _prog = _build()
    x = np.asarray(x, np.float32)
    mask = np.asarray(mask)
    Wqkv = np.asarray(Wqkv, np.float32)
    bqkv = np.asarray(bqkv, np.float32)
    Wproj = np.asarray(Wproj, np.float32)
    bproj = np.asarray(bproj, np.float32)
    cos, sin = _rope_tables()
    in_maps = [
        _core_inputs(x, mask, Wqkv, Wproj, bqkv, cos, sin, core // 2, core % 2)
        for core in range(8)
    ]
    res = run_bass_kernel_spmd(_prog, in_maps, list(range(8)), trace=trace)
    acc = np.zeros((SEQ, E), np.float64)
    for core in range(8):
        s = core % 2
        acc[SQ * s:SQ * (s + 1)] += res.results[core]["pout"].astype(np.float64)
    bias_row = bproj.astype(np.float64) + Wproj.astype(np.float64) @ \
        bqkv[1536:2304].astype(np.float64)
    acc += bias_row
    return acc.astype(np.float32), res


def kernel(x, mask, Wqkv, bqkv, Wproj, bproj):
    out, _ = _run(x, mask, Wqkv, bqkv, Wproj, bproj, trace=False)
    return out


# revision 13
# speedup vs baseline: 1.5306x; 1.1846x over previous
"""Multi-head attention (2D-RoPE, masked softmax) on 8 Trainium2 NeuronCores.

Sharding: 4 head-groups (3 heads each) x 2 query-halves (1160 rows each).
Each core computes full attention for its 3 heads over its 1160 query rows
against all 2320 keys, plus its share of the output projection; the host
sums the 8 partial projections and adds the (folded) biases.

Device-side layout notes:
  - Everything PE-facing is float32r (~13-bit mantissa, 1 cyc/row at N>=256).
  - K/Q are produced directly in transposed [head_dim, seq] layout so
    scores come out as scoresT[m, l] with keys on partitions -> softmax
    needs no partition reduction: exp on ScalarE straight out of PSUM
    (0.125 scale folded in; no max-subtraction needed, |s*scale| < ~3),
    denominator via a ones-column appended to V (free PSUM row), and the
    mask folded into V (zeroed rows + mask-valued ones-column).
  - Softmax normalization: Z row -> reciprocal_approx_accurate -> K=1
    outer-product matmul broadcasts 1/Z across partitions -> one DVE
    multiply normalizes ctxT before the output projection.
  - Key order (m) is contraction-internal, so each core's x/K-tables/mask
    are permuted to put its own query rows first; one xT buffer serves
    both the K/V projections (all columns) and the Q projection (first
    1160 columns), keeping the program identical across cores (SPMD).
  - V-bias and output bias never touch the device:
    out = softmax(..) @ (Vx + bv) @ Wp.T + bp = dev_out + (Wp @ bv + bp).
"""
import sys
if '/opt/trn_rl_repo' not in sys.path:
    sys.path.insert(0, '/opt/trn_rl_repo')
import numpy as np

SEQ, E, NH, D = 2320, 768, 12, 64
GRID, TASK = 48, 16
SQ = SEQ // 2           # query rows per core
HG = 3                  # heads per core
SCALE = D ** -0.5
EC = 6                  # embed chunks of 128
L_TILES = [(0, 512), (512, 392), (904, 256)]
N_TILES = [(0, 512), (512, 512), (1024, 512), (1536, 512), (2048, 272)]
MC = [(i * 128, min(128, SEQ - i * 128)) for i in range(19)]
PT = [(i * 128, min(128, SQ - i * 128)) for i in range(10)]

_prog = None


def _build(stages=3):
    import concourse.mybir as mybir
    import concourse.tile as tile
    from concourse import bacc

    F32, F32R = mybir.dt.float32, mybir.dt.float32r
    AF = mybir.ActivationFunctionType

    nc = bacc.Bacc('TRN2', target_bir_lowering=False, debug=False, num_devices=8)
    dp = nc.declare_dram_parameter
    xt_d = dp("xt", [E, SEQ], F32R, isOutput=False)
    wq_d = dp("wq", [E, 192], F32R, isOutput=False)
    wk_d = dp("wk", [E, 192], F32R, isOutput=False)
    wv_d = dp("wv", [E, 256], F32R, isOutput=False)
    wp_d = dp("wp", [192, E], F32R, isOutput=False)
    bq_d = dp("bq", [128, 2], F32, isOutput=False)
    bk_d = dp("bk", [128, 2], F32, isOutput=False)
    cq_d = dp("cq", [128, SQ], F32, isOutput=False)
    sq_d = dp("sq", [128, SQ], F32, isOutput=False)
    ck_d = dp("ck", [128, SEQ], F32, isOutput=False)
    sk_d = dp("sk", [128, SEQ], F32, isOutput=False)
    mk_d = dp("mk", [19, 128], F32, isOutput=False)
    on_d = dp("ones64", [1, 64], F32R, isOutput=False)
    out_d = dp("pout", [SQ, E], F32, isOutput=True)

    with tile.TileContext(nc) as tc:
        with (
            tc.tile_pool(name="long", bufs=1) as lp,
            tc.tile_pool(name="zp", bufs=2) as zp,
        ):
            kt_h = [lp.tile([128, SEQ], F32R, tag=f"kt{h}", name=f"kt{h}")
                    for h in range(HG)]
            qt_h = [lp.tile([128, SQ], F32R, tag=f"qt{h}", name=f"qt{h}")
                    for h in range(HG)]
            v_all = lp.tile([128, 19, HG, 65], F32R, tag="v_all")
            wp_h = [lp.tile([128, E], F32R, tag=f"wp{h}", name=f"wp{h}") for h in range(HG)]
            ctxn_h = [lp.tile([128, SQ], F32R, tag=f"ctxn{h}", name=f"ctxn{h}") for h in range(HG)]
            ones64 = lp.tile([1, 64], F32R, tag="ones64")
            mk_sb = lp.tile([128, 19], F32, tag="mk")

            for h in range(HG):
                nc.sync.dma_start(wp_h[h][0:64, :], wp_d[64 * h:64 * (h + 1), :])
                nc.gpsimd.memset(wp_h[h][64:128, :].bitcast(F32), 0.0)
                nc.gpsimd.memset(kt_h[h][64:128, :].bitcast(F32), 0.0)
                nc.gpsimd.memset(qt_h[h][64:128, :].bitcast(F32), 0.0)
                nc.gpsimd.memset(ctxn_h[h][64:128, :].bitcast(F32), 0.0)
            nc.sync.dma_start(ones64[:], on_d[:])
            nc.sync.dma_start(mk_sb[:], mk_d[:].rearrange("t p -> p t"))

            with tc.tile_pool(name="p12", bufs=1) as p12, \
                 tc.tile_pool(name="rp", bufs=2) as rp, \
                 tc.tile_pool(name="pk", bufs=2, space="PSUM") as pkp, \
                 tc.tile_pool(name="pv", bufs=2, space="PSUM") as pvp:
                xt = []
                for c in range(EC):
                    t = p12.tile([128, SEQ], F32R, tag=f"xt{c}", name=f"xt{c}")
                    for (off, n) in N_TILES:
                        nc.sync.dma_start(t[:, off:off + n],
                                          xt_d[c * 128:(c + 1) * 128, off:off + n])
                    xt.append(t)
                wq_sb = p12.tile([128, EC, 192], F32R, tag="wq")
                nc.sync.dma_start(wq_sb[:], wq_d[:].rearrange("(c p) n -> p c n", p=128))
                wk_sb = p12.tile([128, EC, 192], F32R, tag="wk")
                nc.sync.dma_start(wk_sb[:], wk_d[:].rearrange("(c p) n -> p c n", p=128))
                wv_sb = p12.tile([128, EC, 256], F32R, tag="wv")
                nc.sync.dma_start(wv_sb[:], wv_d[:].rearrange("(c p) n -> p c n", p=128))
                bq_sb = p12.tile([128, 2], F32, tag="bq")
                nc.sync.dma_start(bq_sb[:], bq_d[:])
                bk_sb = p12.tile([128, 2], F32, tag="bk")
                nc.sync.dma_start(bk_sb[:], bk_d[:])
                cq_sb = p12.tile([128, SQ], F32, tag="cq")
                nc.sync.dma_start(cq_sb[:], cq_d[:])
                sq_sb = p12.tile([128, SQ], F32, tag="sq")
                nc.sync.dma_start(sq_sb[:], sq_d[:])
                ck_sb = p12.tile([128, SEQ], F32, tag="ck")
                nc.sync.dma_start(ck_sb[:], ck_d[:])
                sk_sb = p12.tile([128, SEQ], F32, tag="sk")
                nc.sync.dma_start(sk_sb[:], sk_d[:])

                def qk_proj_tile(w_sb, b_sb, cos_sb, sin_sb, h, off, n, outt):
                    # one head, one n-tile: project (6-chunk accum), add bias,
                    # rope, write rows 0:64 of the padded output tile.
                    brow, bcol = (h % 2) * 64, h // 2
                    ps = pkp.tile([64, 512], F32, tag="pk", name="pk")
                    for c in range(EC):
                        nc.tensor.matmul(
                            ps[0:64, 0:n],
                            w_sb[:, c, h * 64:(h + 1) * 64],
                            xt[c][:, off:off + n],
                            start=(c == 0), stop=(c == EC - 1),
                        )
                    raw = rp.tile([64, 512], F32, tag="raw", name="raw")
                    nc.vector.tensor_scalar_add(
                        raw[0:64, 0:n], ps[0:64, 0:n],
                        b_sb[brow:brow + 64, bcol:bcol + 1])
                    t1 = rp.tile([64, 512], F32, tag="rt1", name="rt1")
                    t2 = rp.tile([64, 512], F32, tag="rt2", name="rt2")
                    nc.vector.tensor_mul(
                        t1[0:64, 0:n], raw[0:64, 0:n], cos_sb[0:64, off:off + n])
                    for b in range(2):
                        src = b * 32 + (32 if b % 2 == 0 else -32)
                        nc.vector.tensor_mul(
                            t2[b * 32:(b + 1) * 32, 0:n],
                            raw[src:src + 32, 0:n],
                            sin_sb[src:src + 32, off:off + n])
                    nc.vector.tensor_add(
                        outt[0:64, off:off + n], t1[0:64, 0:n], t2[0:64, 0:n])

                def v_tile(i):
                    off, m = MC[i]
                    pv = pvp.tile([128, 256], F32, tag="pv", name="pv")
                    for c in range(EC):
                        nc.tensor.matmul(
                            pv[0:m, :], xt[c][:, off:off + m], wv_sb[:, c, :],
                            start=(c == 0), stop=(c == EC - 1))
                    nc.vector.tensor_mul(
                        v_all[0:m, i, :, 0:64],
                        pv[0:m, 0:192].rearrange("p (h d) -> p h d", h=HG),
                        mk_sb[0:m, i:i + 1].to_broadcast([m, HG, 64]))
                    nc.vector.tensor_copy(
                        v_all[0:m, i, :, 64:65],
                        mk_sb[0:m, i:i + 1].to_broadcast([m, HG, 1]))

                jobs = []
                for h in range(HG):
                    for (off, n) in N_TILES:
                        jobs.append(("k", h, off, n))
                    for (off, n) in L_TILES:
                        jobs.append(("q", h, off, n))
                vjobs = [("v", i) for i in range(len(MC))]
                mixed = []
                vi = 0
                for j, job in enumerate(jobs):
                    mixed.append(job)
                    while vi * len(jobs) < (j + 1) * len(vjobs):
                        mixed.append(vjobs[vi])
                        vi += 1
                for job in mixed:
                    if job[0] == "v":
                        v_tile(job[1])
                    elif job[0] == "k":
                        _, h, off, n = job
                        qk_proj_tile(wk_sb, bk_sb, ck_sb, sk_sb, h, off, n, kt_h[h])
                    else:
                        _, h, off, n = job
                        qk_proj_tile(wq_sb, bq_sb, cq_sb, sq_sb, h, off, n, qt_h[h])

            if stages < 2:
                nc.sync.dma_start(out_d[0:128, :],
                                  kt_h[0][0:128, 0:768].bitcast(F32))
            # ---- attention phase ----
            with tc.tile_pool(name="p3", bufs=1) as p3, \
                 tc.tile_pool(name="ep", bufs=2) as ep, \
                 tc.tile_pool(name="op", bufs=2) as op, \
                 tc.tile_pool(name="rzp", bufs=2) as rzp, \
                 tc.tile_pool(name="ps3", bufs=2, space="PSUM") as ps3, \
                 tc.tile_pool(name="pc3", bufs=2, space="PSUM") as pc3, \
                 tc.tile_pool(name="pz3", bufs=1, space="PSUM") as pz3, \
                 tc.tile_pool(name="pp3", bufs=1, space="PSUM") as pp3:
                for h in range(HG if stages >= 2 else 0):
                    ktap, qtap = kt_h[h], qt_h[h]
                    ctxap = ctxn_h[h]
                    for (loff, ln) in L_TILES:
                        pctx = pc3.tile([65, 512], F32, tag="pctx")
                        # software-pipelined over PAIRS of m-chunks: one exp
                        # op covers two chunks (striding across two psum
                        # banks), halving ScalarE op count; ctx matmuls for
                        # pair p issue after scores of pair p+1, so exp
                        # overlaps PE work.
                        PAIRS = [(i, i + 1) if i + 1 < len(MC) else (i,)
                                 for i in range(0, len(MC), 2)]
                        exs = {}

                        def scores_exp(p):
                            chunks = PAIRS[p]
                            ps = ps3.tile([128, 1024], F32, tag="ps", name="ps")
                            for j, i in enumerate(chunks):
                                moff, m = MC[i]
                                kk = 128 if m == 128 else 64
                                nc.tensor.matmul(
                                    ps[0:m, j * 512:j * 512 + ln],
                                    ktap[0:kk, moff:moff + m],
                                    qtap[0:kk, loff:loff + ln],
                                    start=True, stop=True)
                            ex = ep.tile([128, 2, 512], F32R, tag="ex", name="ex")
                            m0 = MC[chunks[0]][1]
                            if len(chunks) == 2:
                                nc.scalar.activation(
                                    ex[0:m0, :, 0:ln],
                                    ps[0:m0, :].rearrange(
                                        "p (two n) -> p two n", two=2)[:, :, 0:ln],
                                    AF.Exp, bias=0.0, scale=SCALE)
                            else:
                                nc.scalar.activation(
                                    ex[0:m0, 0, 0:ln], ps[0:m0, 0:ln], AF.Exp,
                                    bias=0.0, scale=SCALE)
                            exs[p] = ex

                        def ctx_mm(p):
                            ex = exs.pop(p)
                            for j, i in enumerate(PAIRS[p]):
                                moff, m = MC[i]
                                nc.tensor.matmul(
                                    pctx[:, 0:ln], v_all[0:m, i, h, :],
                                    ex[0:m, j, 0:ln],
                                    start=(i == 0), stop=(i == len(MC) - 1))

                        for p in range(len(PAIRS) + 1):
                            if p < len(PAIRS):
                                scores_exp(p)
                            if p >= 1:
                                ctx_mm(p - 1)
                        # softmax denominator -> broadcast -> normalize
                        zrow = zp.tile([1, 512], F32, tag="zrow")
                        nc.scalar.copy(zrow[0:1, 0:ln], pctx[64:65, 0:ln])
                        zscr = zp.tile([1, 512], F32, tag="zscr")
                        rzf = zp.tile([1, 512], F32, tag="rzf")
                        nc.vector.reciprocal_approx_accurate(
                            rzf[0:1, 0:ln], zrow[0:1, 0:ln], zscr[0:1, 0:ln])
                        rzr = zp.tile([1, 512], F32R, tag="rzr")
                        nc.vector.tensor_copy(rzr[0:1, 0:ln], rzf[0:1, 0:ln])
                        przb = pz3.tile([64, 512], F32, tag="przb")
                        nc.tensor.matmul(
                            przb[:, 0:ln], ones64[:], rzr[0:1, 0:ln],
                            start=True, stop=True)
                        rzb = rzp.tile([64, 512], F32, tag="rzb")
                        nc.scalar.copy(rzb[:, 0:ln], przb[:, 0:ln])
                        nc.vector.tensor_mul(
                            ctxap[0:64, loff:loff + ln],
                            pctx[0:64, 0:ln], rzb[0:64, 0:ln])

                if stages == 2:
                    nc.sync.dma_start(out_d[0:64, :],
                                      ctxn_h[0][0:64, 0:768].bitcast(F32))
                # output projection: 3 head-chunks accumulate per l-slice
                for (toff, tm) in (PT if stages >= 3 else []):
                    outsb = op.tile([128, E], F32, tag="outsb")
                    for half in range(2):
                        hs = half * 384
                        pp = pp3.tile([128, 384], F32, tag="pp")
                        for hh in range(HG):
                            nc.tensor.matmul(
                                pp[0:tm, :], ctxn_h[hh][0:128, toff:toff + tm],
                                wp_h[hh][0:128, hs:hs + 384],
                                start=(hh == 0), stop=(hh == HG - 1))
                        nc.vector.tensor_copy(outsb[0:tm, hs:hs + 384], pp[0:tm, :])
                    nc.sync.dma_start(out_d[toff:toff + tm, :], outsb[0:tm, :])
    nc.finalize()
    return nc


def _rope_tables():
    dim = D // 2
    freqs = 1.0 / 10000 ** (np.arange(0, dim, 2, dtype=np.float64) / dim)
    t = np.arange(GRID, dtype=np.float64)
    f = np.repeat(np.outer(t, freqs), 2, axis=-1)                  # [48, 32]
    fr = np.broadcast_to(f[:, None, :], (GRID, GRID, dim))
    fc = np.broadcast_to(f[None, :, :], (GRID, GRID, dim))
    full = np.concatenate([fr, fc], axis=-1).reshape(GRID * GRID, D)
    cos = np.ones((SEQ, D), np.float64)
    sin = np.zeros((SEQ, D), np.float64)
    cos[TASK:] = np.cos(full)
    sin[TASK:] = np.sin(full)
    return cos.astype(np.float32), sin.astype(np.float32)


def _signed_stack(tT):
    # [64, S] -> [128, S]: signed sine table stored at the ROTATED (source)
    # rows, so the rope half-multiplies read both operands at equal partition
    # bases: sinB[32:64] = -sin[0:32], sinB[0:32] = +sin[32:64], stacked x2.
    s = np.vstack([tT[32:64], -tT[0:32]])
    return np.ascontiguousarray(np.vstack([s, s]))


def _core_inputs(x, mask, Wqkv, Wproj, bqkv, cos, sin, g, s):
    xT = x.T  # [768, 2320]
    q0 = SQ * s
    if s == 0:
        perm = None
        xt = np.ascontiguousarray(xT)
    else:
        perm = np.concatenate([np.arange(SQ, SEQ), np.arange(0, SQ)])
        xt = np.ascontiguousarray(np.concatenate([xT[:, SQ:], xT[:, :SQ]], axis=1))
    r0 = 192 * g
    wq = np.ascontiguousarray(Wqkv[r0:r0 + 192, :].T)
    wk = np.ascontiguousarray(Wqkv[768 + r0:768 + r0 + 192, :].T)
    wv = np.zeros((E, 256), np.float32)
    wv[:, 0:192] = Wqkv[1536 + r0:1536 + r0 + 192, :].T
    wp = np.ascontiguousarray(Wproj[:, r0:r0 + 192].T)
    bq = np.zeros((128, 2), np.float32)
    bq[:, 0] = bqkv[r0:r0 + 128]
    bq[0:64, 1] = bqkv[r0 + 128:r0 + 192]
    bk = np.zeros((128, 2), np.float32)
    bk[:, 0] = bqkv[768 + r0:768 + r0 + 128]
    bk[0:64, 1] = bqkv[768 + r0 + 128:768 + r0 + 192]
    cosT, sinT = cos.T, sin.T  # [64, S]
    cq = np.ascontiguousarray(np.vstack([cosT, cosT])[:, q0:q0 + SQ])
    sq = np.ascontiguousarray(_signed_stack(sinT)[:, q0:q0 + SQ])
    ckf = np.vstack([cosT, cosT])
    skf = _signed_stack(sinT)
    if perm is not None:
        ckf = ckf[:, perm]
        skf = skf[:, perm]
    mk = mask.astype(np.float32)
    if perm is not None:
        mk = mk[perm]
    mk = np.concatenate([mk, np.zeros(19 * 128 - SEQ, np.float32)]).reshape(19, 128)
    return {
        "xt": xt, "wq": wq, "wk": wk, "wv": wv, "wp": wp,
        "bq": bq, "bk": bk,
        "cq": cq, "sq": sq,
        "ck": np.ascontiguousarray(ckf), "sk": np.ascontiguousarray(skf),
        "mk": np.ascontiguousarray(mk),
        "ones64": np.ones((1, 64), np.float32),
    }


def _run(x, mask, Wqkv, bqkv, Wproj, bproj, trace=False):
    global _prog
    from concourse.bass_utils import run_bass_kernel_spmd
    if _prog is None:
        _prog = _build()
    x = np.asarray(x, np.float32)
    mask = np.asarray(mask)
    Wqkv = np.asarray(Wqkv, np.float32)
    bqkv = np.asarray(bqkv, np.float32)
    Wproj = np.asarray(Wproj, np.float32)
    bproj = np.asarray(bproj, np.float32)
    cos, sin = _rope_tables()
    in_maps = [
        _core_inputs(x, mask, Wqkv, Wproj, bqkv, cos, sin, core // 2, core % 2)
        for core in range(8)
    ]
    res = run_bass_kernel_spmd(_prog, in_maps, list(range(8)), trace=trace)
    acc = np.zeros((SEQ, E), np.float64)
    for core in range(8):
        s = core % 2
        acc[SQ * s:SQ * (s + 1)] += res.results[core]["pout"].astype(np.float64)
    bias_row = bproj.astype(np.float64) + Wproj.astype(np.float64) @ \
        bqkv[1536:2304].astype(np.float64)
    acc += bias_row
    return acc.astype(np.float32), res


def kernel(x, mask, Wqkv, bqkv, Wproj, bproj):
    out, _ = _run(x, mask, Wqkv, bqkv, Wproj, bproj, trace=False)
    return out
